# revision 39
# baseline (speedup 1.0000x reference)
"""Multi-head attention (B=2, S=2048, D=1024, H=16) on 8 Trainium2 cores.

Sharding: core i -> batch i//4, head-group i%4 (4 heads = 2 pairs of 2).
v4: proj order q->k->v so the gapless 128x1038ns exp stream (the hard ACT
floor of the attention phase) starts right after the k projection (~36us)
instead of after v.  The v projection runs as 8 sequential 2-chunk rounds
in the two vup PSUM banks (one accumulation group per bank), interleaved
into blocks 0-2 of the attention phase as xv streams in.  attnV for block
n runs during block n+2 (probs buffered bf16 in a 36-slot ring), norm(n)
at block n+3 (1/Z copies move to ACT once the exp stream has ended), and
the output projection for query-block Q accumulates BOTH head pairs into
one PSUM tile during block 2Q+4, emitting a single bf16 partial per core
(half the output DMA).  Bias adds are split ACT/DVE, pair 0 first, so the
first scores fire as soon as the k psum ring drains; the Exp table is
preloaded at t=0.  Host sums 8 partials and adds (bv @ Wo + bo) once.
"""

import sys

import numpy as np

try:
    import concourse.bacc as bacc
except ImportError:  # grading dir may not have the repo on sys.path
    sys.path.insert(0, "/opt/trn_rl_repo")
    import concourse.bacc as bacc

import ml_dtypes
import concourse.mybir as mybir
import concourse.tile as tile
from concourse import bass_utils

B, S, D, H, DH = 2, 2048, 1024, 16, 64
F32 = mybir.dt.float32
R32 = mybir.dt.float32r
XDT = mybir.dt.bfloat16  # dtype of streamed x, wq/wk/wv, probs, out
EXP = mybir.ActivationFunctionType.Exp

NBLK = 8    # real blocks: 2 pairs x 4 query-blocks of 512
NSLOT = 11  # block slots incl. drain-out phantoms


def _emit(nc, aps):
    xq, xk, xv = aps["xqT"], aps["xkT"], aps["xvT"]
    out_ap = aps["out"]

    with tile.TileContext(nc) as tc, \
         nc.allow_low_precision(reason="bf16/fp32r matmul input pipeline"):
        with tc.tile_pool(name="persist", bufs=1, space="SBUF") as sb, \
             tc.tile_pool(name="xres", bufs=8, space="SBUF") as xvp, \
             tc.tile_pool(name="xstream", bufs=4, space="SBUF") as xp, \
             tc.tile_pool(name="pexp", bufs=36, space="SBUF") as pa_pool, \
             tc.tile_pool(name="zpool", bufs=2, space="SBUF") as z_pool, \
             tc.tile_pool(name="bpool", bufs=2, space="SBUF") as bsc_pool, \
             tc.tile_pool(name="obpool", bufs=6, space="SBUF") as ob_pool:

            wq_sb = sb.tile([128, 2048], XDT)
            wk_sb = sb.tile([128, 2048], XDT)
            wv_sb = sb.tile([128, 2048], XDT)
            wo_sb = sb.tile([128, 2048], R32)
            bqT_sb = sb.tile([128, 2], F32)
            bkT_sb = sb.tile([128, 2], F32)
            ones_sb = sb.tile([128, 64], R32)
            qT_sb = sb.tile([128, 4096], R32)
            kT_sb = sb.tile([128, 4096], R32)
            attnT_sb = sb.tile([128, 1024], R32)
            vaug_sb = sb.tile([128, 16 * 260], XDT)

            # memset cannot target fp32r; stage 1.0s through an F32 tile.
            onesF = sb.tile([128, 64], F32)
            nc.vector.memset(onesF[:], 1.0)
            nc.vector.tensor_copy(ones_sb[:], onesF[:])
            # preload the Exp table off the critical path (the implicit
            # table load would otherwise serialize before the first real exp)
            warm = sb.tile([128, 1], F32)
            nc.scalar.activation(warm[:], onesF[:, 0:1], EXP)
            # v_aug row layout per key-chunk j (260 cols): 4 x [v(64) 1]
            vj = vaug_sb[:].rearrange("p (j r) -> p j r", r=260)
            for c in (64, 129, 194, 259):
                nc.vector.tensor_copy(vj[:, :, c:c + 1],
                                      onesF[:, 0:16].unsqueeze(2))
            # single-descriptor-batch weight loads: one dma_start each
            nc.sync.dma_start(
                wq_sb[:].rearrange("p (d c) -> p d c", c=256),
                aps["wq"][:].rearrange("(d p) c -> p d c", p=128))
            nc.sync.dma_start(bqT_sb[:], aps["bqT"][:])

            # ---- q then k projections share one 8-bank PSUM ring ----
            with tc.tile_pool(name="projp", bufs=8, space="PSUM") as pp:
                qps = [pp.tile([128, 512], F32, tag="pp", name=f"qp{i}")
                       for i in range(8)]
                # dummy matmuls ramp the PE clock (0.65->2.4GHz over ~3us)
                for i in range(10):
                    nc.tensor.matmul(qps[0][0:64, 0:64], onesF[:, 0:64],
                                     onesF[:, 0:64], start=True, stop=True)
                for dc in range(8):
                    xt = xp.tile([128, 2048], XDT, tag="xs", name=f"xq{dc}")
                    # slice-level DMAs: the first matmul of the pass only
                    # waits one 512-col slice (+sem prop), not the full chunk
                    for h2 in range(2):
                        nc.sync.dma_start(
                            xt[:, h2 * 1024:(h2 + 1) * 1024],
                            xq[dc * 128:(dc + 1) * 128, h2 * 1024:(h2 + 1) * 1024])
                    for sc in range(4):
                        for cc in range(2):
                            nc.tensor.matmul(
                                qps[cc * 4 + sc][:],
                                wq_sb[:, dc * 256 + cc * 128:dc * 256 + cc * 128 + 128],
                                xt[:, sc * 512:(sc + 1) * 512],
                                start=(dc == 0), stop=(dc == 7))
                nc.sync.dma_start(
                    wk_sb[:].rearrange("p (d c) -> p d c", c=256),
                    aps["wk"][:].rearrange("(d p) c -> p d c", p=128))
                nc.sync.dma_start(bkT_sb[:], aps["bkT"][:])
                for cc in range(2):
                    for sc in range(4):
                        dst = qT_sb[:, cc * 2048 + sc * 512:cc * 2048 + sc * 512 + 512]
                        if sc % 2 == 0:
                            nc.scalar.add(dst, qps[cc * 4 + sc][:],
                                          bqT_sb[:, cc:cc + 1])
                        else:
                            nc.vector.tensor_scalar_add(
                                dst, qps[cc * 4 + sc][:], bqT_sb[:, cc:cc + 1])

                kps = [pp.tile([128, 512], F32, tag="pp", name=f"kp{i}")
                       for i in range(8)]
                for dc in range(8):
                    xt = xp.tile([128, 2048], XDT, tag="xs", name=f"xk{dc}")
                    for h2 in range(2):
                        nc.sync.dma_start(
                            xt[:, h2 * 1024:(h2 + 1) * 1024],
                            xk[dc * 128:(dc + 1) * 128, h2 * 1024:(h2 + 1) * 1024])
                    # final pass pair-0 first: its psum tiles close early
                    # so the cc0 bias adds (gating the first scores) start
                    order = ([(cc, sc) for cc in range(2) for sc in range(4)]
                             if dc == 7 else
                             [(cc, sc) for sc in range(4) for cc in range(2)])
                    for cc, sc in order:
                        nc.tensor.matmul(
                            kps[cc * 4 + sc][:],
                            wk_sb[:, dc * 256 + cc * 128:dc * 256 + cc * 128 + 128],
                            xt[:, sc * 512:(sc + 1) * 512],
                            start=(dc == 0), stop=(dc == 7))
                nc.sync.dma_start(
                    wv_sb[:].rearrange("p (d c) -> p d c", c=256),
                    aps["wv"][:].rearrange("(d p) c -> p d c", p=128))
                # k bias adds gate the attention psum region: split them
                # across ACT+DVE, pair 0 (cc=0, the sp banks) first
                for cc in range(2):
                    for sc in range(4):
                        dst = kT_sb[:, cc * 2048 + sc * 512:cc * 2048 + sc * 512 + 512]
                        if sc % 2 == 0:
                            nc.scalar.add(dst, kps[cc * 4 + sc][:],
                                          bkT_sb[:, cc:cc + 1])
                        else:
                            nc.vector.tensor_scalar_add(
                                dst, kps[cc * 4 + sc][:], bkT_sb[:, cc:cc + 1])

            # xv streams into an 8-slot resident ring (consumed by the
            # interleaved v projection during blocks 0-1)
            xvt = []
            for dc in range(8):
                xt = xvp.tile([128, 2048], XDT, tag="xv", name=f"xv{dc}")
                nc.sync.dma_start(xt[:], xv[dc * 128:(dc + 1) * 128, :])
                xvt.append(xt)
            for p_ in range(2):
                nc.sync.dma_start(wo_sb[:, p_ * 1024:(p_ + 1) * 1024],
                                  aps["wo"][p_ * 128:(p_ + 1) * 128, :])

            # ---- attention phase: scores/exp + interleaved v proj,
            #      lag-2 attnV, lag-3 norm, outproj(Q) at block 2Q+4 ----
            with tc.tile_pool(name="sp", bufs=2, space="PSUM") as sp, \
                 tc.tile_pool(name="opp", bufs=2, space="PSUM") as opp, \
                 tc.tile_pool(name="vup", bufs=2, space="PSUM") as vup:

                # v projection: 8 sequential rounds of 2 key-chunks, each
                # chunk accumulating in its own vup bank (one PSUM group per
                # bank at a time).  Round 0 is xv-arrival-paced across block
                # 0; rounds 1-7 are spread 3 passes/chunk over block 1 and
                # block 2; each round's chunks are evacuated right before
                # the bank's next round starts.
                vtile = {}

                def valloc(r):
                    for ci in range(2):
                        c = 2 * r + ci
                        vtile[c] = vup.tile([128, 256], F32, tag="v",
                                            name=f"vt{c}")

                def vpass(r, dc):
                    if dc == 0:
                        valloc(r)
                    for ci in range(2):
                        c = 2 * r + ci
                        nc.tensor.matmul(
                            vtile[c][:],
                            xvt[dc][:, c * 128:(c + 1) * 128],
                            wv_sb[:, dc * 256:(dc + 1) * 256],
                            start=(dc == 0), stop=(dc == 7))

                def vevac(c):
                    base = c * 260
                    dst = vaug_sb[:, base:base + 260] \
                        .rearrange("p (g c) -> p g c", c=65)[:, :, 0:64]
                    nc.vector.tensor_copy(
                        dst, vtile[c][:].rearrange("p (g c) -> p g c", c=64))

                # action table: (n, j) -> list of ("evac", c) | ("pass", r, dc)
                # r0 dc0-3 are emitted before the block loop (xv is already
                # streaming while the k projection finishes)
                vact = {}
                for dc in range(8):
                    vact.setdefault((0, 2 * dc), []).append(("pass", 0, dc))
                slots = [(1, j) for j in range(16) for _ in range(3)] + \
                        [(2, j) for j in range(8)]
                idx = 0
                vact.setdefault((1, 0), []).append(("evac", 0))
                vact.setdefault((1, 0), []).append(("evac", 1))
                for r in range(1, 8):
                    for dc in range(8):
                        vact.setdefault(slots[idx], []).append(("pass", r, dc))
                        idx += 1
                    nxt = slots[min(idx, len(slots) - 1)] if r < 7 else (2, 8)
                    vact.setdefault(nxt, []).append(("evac", 2 * r))
                    vact.setdefault(nxt, []).append(("evac", 2 * r + 1))

                pBig = {}
                po = {}

                def scores(n, j):
                    Q, p = divmod(n, 2)
                    qb = p * 2048 + Q * 512
                    kb = p * 2048 + j * 128
                    sBig = sp.tile([128, 1024], F32, tag="s", name=f"s{n}_{j}")
                    nc.tensor.matmul(sBig[:, 0:512],
                                     kT_sb[0:64, kb:kb + 128],
                                     qT_sb[0:64, qb:qb + 512],
                                     start=True, stop=True)
                    nc.tensor.matmul(sBig[:, 512:1024],
                                     kT_sb[64:128, kb:kb + 128],
                                     qT_sb[64:128, qb:qb + 512],
                                     start=True, stop=True)
                    pBig[(n, j)] = pa_pool.tile([128, 1024], XDT, tag="pa",
                                                name=f"pb{n}_{j}")
                    nc.scalar.activation(pBig[(n, j)][:], sBig[:], EXP,
                                         scale=0.125)

                def attnv(m, j):
                    _, p = divmod(m, 2)
                    va = j * 260 + 2 * p * 65
                    nc.tensor.matmul(po[m][0][0:65, 0:512],
                                     vaug_sb[:, va:va + 65],
                                     pBig[(m, j)][:, 0:512],
                                     start=(j == 0), stop=(j == 15))
                    nc.tensor.matmul(po[m][1][0:65, 0:512],
                                     vaug_sb[:, va + 65:va + 130],
                                     pBig[(m, j)][:, 512:1024],
                                     start=(j == 0), stop=(j == 15))

                def norm(m, tail=False):
                    _, p = divmod(m, 2)
                    poA, poB = po[m]
                    z = z_pool.tile([128, 1024], R32, tag="z", name=f"z{m}")
                    nc.vector.reciprocal(z[64:65, 0:512], poA[64:65, 0:512])
                    nc.vector.reciprocal(z[64:65, 512:1024], poB[64:65, 0:512])
                    zzA = vup.tile([128, 512], F32, tag="v", name=f"zzA{m}")
                    nc.tensor.matmul(zzA[0:64, 0:512], ones_sb[64:65, :],
                                     z[64:65, 0:512], start=True, stop=True)
                    zzB = vup.tile([128, 512], F32, tag="v", name=f"zzB{m}")
                    nc.tensor.matmul(zzB[0:64, 0:512], ones_sb[64:65, :],
                                     z[64:65, 512:1024], start=True, stop=True)
                    # DVE tensor_tensor cannot read two PSUM operands;
                    # stage the 1/Z broadcast through SBUF (on ACT in the
                    # tail, where the exp stream has ended).
                    cp = nc.scalar.copy if tail else nc.vector.tensor_copy
                    zbsA = z_pool.tile([64, 512], F32, tag="zbs",
                                       name=f"zbA{m}")
                    cp(zbsA[:], zzA[0:64, 0:512])
                    nc.vector.tensor_mul(attnT_sb[0:64, p * 512:p * 512 + 512],
                                         poA[0:64, 0:512], zbsA[:])
                    zbsB = z_pool.tile([64, 512], F32, tag="zbs",
                                       name=f"zbB{m}")
                    cp(zbsB[:], zzB[0:64, 0:512])
                    bsc = bsc_pool.tile([64, 512], R32, tag="b", name=f"bs{m}")
                    nc.vector.tensor_mul(bsc[:], poB[0:64, 0:512], zbsB[:])
                    nc.sync.dma_start(attnT_sb[64:128, p * 512:p * 512 + 512],
                                      bsc[:])

                def upstep(Q, idx, act_evac=False, pool=None):
                    t, nn_ = divmod(idx, 2)
                    pl, tg = (pool or vup), ("o" if pool is opp else "v")
                    up = pl.tile([128, 512], F32, tag=tg, name=f"u{Q}_{idx}")
                    nc.tensor.matmul(
                        up[:],
                        attnT_sb[:, t * 128:t * 128 + 128],
                        wo_sb[:, nn_ * 512:nn_ * 512 + 512],
                        start=True, stop=False)
                    nc.tensor.matmul(
                        up[:],
                        attnT_sb[:, 512 + t * 128:512 + t * 128 + 128],
                        wo_sb[:, 1024 + nn_ * 512:1024 + nn_ * 512 + 512],
                        start=False, stop=True)
                    ob = ob_pool.tile([128, 512], XDT, tag="ob",
                                      name=f"ob{Q}_{idx}")
                    if act_evac:
                        nc.scalar.copy(ob[:], up[:])
                    else:
                        nc.vector.tensor_copy(ob[:], up[:])
                    nc.sync.dma_start(
                        out_ap[Q * 512 + t * 128:Q * 512 + t * 128 + 128,
                               nn_ * 512:(nn_ + 1) * 512],
                        ob[:])

                for n in range(NSLOT):
                    for j in range(16):
                        if n < NBLK:
                            scores(n, j)
                        for act in vact.get((n, j), ()):
                            if act[0] == "pass":
                                vpass(act[1], act[2])
                            else:
                                vevac(act[1])
                        if j == 0 and 3 <= n:
                            m = n - 3
                            if m < NBLK:
                                norm(m, tail=(n >= 9))
                        m = n - 2
                        if 0 <= m < NBLK:
                            if j == 0:
                                po[m] = (
                                    opp.tile([128, 512], F32, tag="o",
                                             name=f"poA{m}"),
                                    opp.tile([128, 512], F32, tag="o",
                                             name=f"poB{m}"))
                            attnv(m, j)
                        if n >= 4 and n % 2 == 0 and j % 2 == 1:
                            upstep((n - 4) // 2, j // 2,
                                   act_evac=(n == 10 and j % 4 == 1))


_NC = None


def _get_nc():
    global _NC
    if _NC is None:
        nc = bacc.Bacc("TRN2", target_bir_lowering=False, debug=False,
                       enable_asserts=False, num_devices=8)
        aps = {}
        for nm, shp in [("xqT", (D, S)), ("xkT", (D, S)), ("xvT", (D, S)),
                        ("wq", (D, 256)), ("wk", (D, 256)), ("wv", (D, 256))]:
            aps[nm] = nc.dram_tensor(nm, shp, XDT, kind="ExternalInput").ap()
        aps["wo"] = nc.dram_tensor("wo", (256, D), R32, kind="ExternalInput").ap()
        for nm, shp in [("bqT", (128, 2)), ("bkT", (128, 2))]:
            aps[nm] = nc.dram_tensor(nm, shp, F32, kind="ExternalInput").ap()
        aps["out"] = nc.dram_tensor("out", (S, D), XDT, kind="ExternalOutput").ap()
        _emit(nc, aps)
        nc.compile()
        _NC = nc
    return _NC


def _run(inputs, trace=False):
    nc = _get_nc()
    f = np.float32
    bf = ml_dtypes.bfloat16
    q = np.asarray(inputs["query"], dtype=f)
    k = np.asarray(inputs["key"], dtype=f)
    v = np.asarray(inputs["value"], dtype=f)
    Wq = np.asarray(inputs["Wq"], dtype=f)
    Wk = np.asarray(inputs["Wk"], dtype=f)
    Wv = np.asarray(inputs["Wv"], dtype=f)
    Wo = np.asarray(inputs["Wo"], dtype=f)
    bq = np.asarray(inputs["bq"], dtype=f)
    bk = np.asarray(inputs["bk"], dtype=f)
    bv = np.asarray(inputs["bv"], dtype=f)
    bo = np.asarray(inputs["bo"], dtype=f)

    xT = {b: (np.ascontiguousarray(q[b].T).astype(bf),
              np.ascontiguousarray(k[b].T).astype(bf),
              np.ascontiguousarray(v[b].T).astype(bf)) for b in range(B)}
    in_maps = []
    for i in range(8):
        b, hg = divmod(i, 4)
        c0 = hg * 256
        in_maps.append({
            "xqT": xT[b][0], "xkT": xT[b][1], "xvT": xT[b][2],
            "wq": np.ascontiguousarray(Wq[:, c0:c0 + 256]).astype(bf),
            "wk": np.ascontiguousarray(Wk[:, c0:c0 + 256]).astype(bf),
            "wv": np.ascontiguousarray(Wv[:, c0:c0 + 256]).astype(bf),
            "bqT": np.ascontiguousarray(bq[c0:c0 + 256].reshape(2, 128).T),
            "bkT": np.ascontiguousarray(bk[c0:c0 + 256].reshape(2, 128).T),
            "wo": np.ascontiguousarray(Wo[c0:c0 + 256, :]),
        })

    res = bass_utils.run_bass_kernel_spmd(nc, in_maps, core_ids=list(range(8)),
                                          trace=trace)
    out = np.zeros((B, S, D), dtype=f)
    for i in range(8):
        out[i // 4] += np.asarray(res.results[i]["out"]).astype(f)
    out += (bv @ Wo + bo)[None, None, :]
    return out, res


def kernel(**inputs):
    out, _ = _run(inputs, trace=False)
    return out
